# revision 15
# baseline (speedup 1.0000x reference)
"""Trainium2 Bass kernel for nn_AttentionModule_66537633349985 (segment attention pooling).

Math (per graph b): out[b] = sum_n attn_n * emb_n over nodes n with batch[n]==b,
where attn = softmax_b(w_a . tanh(W_c @ emb + b_c) + b_a). The +b_a and the
segment-max subtraction cancel in the softmax ratio, so neither is computed
(scores are bounded by sum|w_a| <= ~11, so exp never overflows in f32).

Sharding: nodes are split evenly across the 8 cores (125000 each); each core
processes 61 whole macro-tiles of 2048 (124928 nodes) and the 72-node tail is
computed exactly on the host. All on-chip work is in H-on-partitions layout
[128, nodes]; the host pre-transposes embeddings once (to bf16 -- halves HBM
traffic; exp/accumulation stay f32).

Device, per 2048-node macro-tile (two-macro software pipeline: slot m emits
the transform of m, the scores of m-1, and the pooling of m-2, so no engine
FIFO waits on a same-slot cross-engine dependency):
    t  = W_c @ embT              bf16 matmul, W stationary (4 x N=512)
    tT = tanh(t + b_c)           ACT, PSUM -> SBUF bf16
    s  = w_a . tT                matmul -> [1,512] rows at partitions 0/32/64/96
    e  = exp(s)                  ACT over the whole PSUM bank (junk rows unused)
    eb = ones (x) e_row          PE rank-1 broadcast to 128 partitions (f32r)
    P[:, j], wE = scalar_tensor_tensor(embT * eb, accum_out)
                                 one fused DVE op per 512-block: the multiply
                                 AND the block sum in a single 1x pass; wE is
                                 a discarded bf16 scratch.
    dump P [128, 4] and e [4, 512] to HBM via SWDGE (gpsimd queue),
    keeping the SP FIFO free to stream embedding prefetches.

Host epilogue: per-segment sums over whole 512-node blocks from P; blocks
containing a segment boundary are recomputed exactly on the host from emb and
the dumped exp(scores); denominators via bincount of dumped exp(scores);
divide and assemble the [1024, 128] output.
"""
import numpy as np

import concourse.bass as bass
import concourse.bacc as bacc
import concourse.tile as tile
import concourse.mybir as mybir
from concourse.bass_utils import run_bass_kernel_spmd

H = 128            # hidden dim
B = 1024           # number of graphs
NCORES = 8
TM = 2048          # nodes per macro-tile
NCH = TM // 512    # 512-node chunks per macro-tile
BLK = 512          # block size of the on-chip partial sums

f32 = mybir.dt.float32
f32r = mybir.dt.float32r
bf16 = mybir.dt.bfloat16

USE_BF16 = True    # embeddings shipped to the device in bf16

_BUILD_CACHE: dict = {}


def build_bass(L: int, repeat: int = 1, bf16_path: bool = True) -> "bacc.Bacc":
    """Per-core Bass program: v1 ops with a two-macro software pipeline.

    Emission per slot m:
      dma(m), W-MMs(m), tanh(m)            # PE + ACT
      scores(m-1)                          # PE (tanh(m-1) long done)
      exp(m-1), e-dump(m-1)                # ACT + SP (scores(m-1) just done)
      bcast(m-2) + STT(m-2), P-dump(m-2)   # PE + DVE
    so no engine FIFO waits on a same-slot cross-engine dependency.
    repeat > 1 replays the pipeline for marginal-time measurement."""
    key = (L, repeat)
    if key in _BUILD_CACHE:
        return _BUILD_CACHE[key]
    assert L % TM == 0
    nmacro = L // TM

    nc = bacc.Bacc("TRN2", target_bir_lowering=False, debug=False)

    embT_d = nc.dram_tensor("embT", [H, L], bf16, kind="ExternalInput")
    W_d = nc.dram_tensor("W", [H, H], bf16, kind="ExternalInput")       # holds W_c.T
    wa_d = nc.dram_tensor("wa", [H, 1], f32, kind="ExternalInput")
    bc_d = nc.dram_tensor("bc", [H, 1], f32, kind="ExternalInput")
    P_d = nc.dram_tensor("P", [nmacro, H, NCH], f32, kind="ExternalOutput")
    e_d = nc.dram_tensor("e", [nmacro, NCH, 512], f32, kind="ExternalOutput")

    Tanh = mybir.ActivationFunctionType.Tanh
    Exp = mybir.ActivationFunctionType.Exp
    seq = [mm for _ in range(repeat) for mm in range(nmacro)]

    with tile.TileContext(nc) as tc:
        with (
            tc.tile_pool(name="const", bufs=1) as cpool,
            tc.tile_pool(name="emb", bufs=10) as epool,
            tc.tile_pool(name="tt", bufs=3) as tpool,
            tc.tile_pool(name="ee", bufs=3) as eepool,
            tc.tile_pool(name="we", bufs=4) as wpool,
            tc.tile_pool(name="pp", bufs=3) as ppool,
            tc.tile_pool(name="pt", bufs=2, space="PSUM") as pt_pool,
            tc.tile_pool(name="ps", bufs=1, space="PSUM") as ps_pool,
            tc.tile_pool(name="pe", bufs=3, space="PSUM") as pe_pool,
        ):
            W_sb = cpool.tile([H, H], bf16)
            wa_sb = cpool.tile([H, 1], f32)
            wa_bf = cpool.tile([H, 1], bf16)
            bc_sb = cpool.tile([H, 1], f32)
            ones_sb = cpool.tile([H, H], f32r)
            nc.sync.dma_start(W_sb[:], W_d[:])
            nc.sync.dma_start(wa_sb[:], wa_d[:])
            nc.sync.dma_start(bc_sb[:], bc_d[:])
            nc.vector.tensor_copy(wa_bf[:], wa_sb[:])
            nc.vector.memset(ones_sb[:].bitcast(f32), 1.0)

            # pipeline registers: stage A (transform done) and B (e ready)
            stA = []   # (m, emb_sb, tT_sb)
            stB = []   # (m, emb_sb, e_sb)

            def emit_transform(m):
                emb_sb = epool.tile([H, TM], bf16, tag="emb", name="emb_sb")
                nc.sync.dma_start(emb_sb[:], embT_d[:, m * TM:(m + 1) * TM])
                tT_sb = tpool.tile([H, TM], bf16, tag="tT", name="tT_sb")
                for h in range(2):
                    psum_t = pt_pool.tile([H, TM // 2], f32, tag="pt",
                                          name="psum_t")
                    for j in range(2):
                        nc.tensor.matmul(
                            psum_t[:, j * 512:(j + 1) * 512],
                            W_sb[:],
                            emb_sb[:, (2 * h + j) * 512:(2 * h + j + 1) * 512],
                            start=True, stop=True,
                        )
                    nc.scalar.activation(
                        tT_sb[:, h * 1024:(h + 1) * 1024], psum_t[:],
                        Tanh, bias=bc_sb[:])
                stA.append((m, emb_sb, tT_sb))

            def emit_scores_exp():
                m, emb_sb, tT_sb = stA.pop(0)
                psum_s = ps_pool.tile([H, 512], f32, tag="ps", name="psum_s")
                for j in range(NCH):
                    nc.tensor.matmul(
                        psum_s[32 * j:32 * j + 1, :],
                        wa_bf[:],
                        tT_sb[:, j * 512:(j + 1) * 512],
                        start=True, stop=True,
                        tile_position=(0, 32 * j),
                    )
                e_sb = eepool.tile([H, 512], f32r, tag="e", name="e_sb")
                nc.scalar.activation(e_sb[:], psum_s[:], Exp)
                nc.gpsimd.dma_start(e_d[m], e_sb[0:H:32, :].bitcast(f32))
                stB.append((m, emb_sb, e_sb))

            def emit_p():
                m, emb_sb, e_sb = stB.pop(0)
                P_sb = ppool.tile([H, NCH], f32, tag="P", name="P_sb")
                for j in range(NCH):
                    psum_eb = pe_pool.tile([H, 512], f32, tag="pe",
                                           name="psum_eb")
                    nc.tensor.matmul(
                        psum_eb[:],
                        ones_sb[32 * j:32 * j + 1, :],
                        e_sb[32 * j:32 * j + 1, :],
                        start=True, stop=True,
                        tile_position=(32 * j, 0),
                    )
                    wE_sb = wpool.tile([H, 512], bf16, tag="wE", name="wE_sb")
                    nc.vector.scalar_tensor_tensor(
                        out=wE_sb[:],
                        in0=emb_sb[:, j * 512:(j + 1) * 512],
                        scalar=1.0,
                        in1=psum_eb[:],
                        op0=mybir.AluOpType.mult,
                        op1=mybir.AluOpType.mult,
                        accum_out=P_sb[:, j:j + 1],
                    )
                nc.gpsimd.dma_start(P_d[m], P_sb[:])

            for m in seq:
                emit_transform(m)
                if len(stA) >= 2:
                    emit_scores_exp()
                if len(stB) >= 2:
                    emit_p()
            while stA:
                emit_scores_exp()
            while stB:
                emit_p()

    nc.compile()
    _BUILD_CACHE[key] = nc
    return nc


def kernel(**inputs) -> np.ndarray:
    emb = np.ascontiguousarray(np.asarray(inputs["embeddings"], dtype=np.float32))
    batch = np.asarray(inputs["batch"]).astype(np.int64)
    W_c = np.asarray(inputs["W_c"], dtype=np.float32)
    b_c = np.asarray(inputs["b_c"], dtype=np.float32)
    w_a = np.asarray(inputs["w_a"], dtype=np.float32)
    # b_a cancels in the softmax; unused.

    N = emb.shape[0]
    assert N % NCORES == 0
    SH = N // NCORES                      # nodes per core
    L = (SH // TM) * TM                   # whole macro-tiles only; the short
    TAIL = SH - L                         # per-core tail is done on the host
    nmacro = L // TM

    import ml_dtypes
    edt_np = ml_dtypes.bfloat16
    embT = np.empty((NCORES, H, L), dtype=edt_np)
    for c in range(NCORES):
        embT[c][:] = emb[c * SH:c * SH + L].T.astype(edt_np)

    nc = build_bass(L)
    Wt = np.ascontiguousarray(W_c.T.astype(edt_np))
    wa_col = np.ascontiguousarray(w_a[:, None])
    bc_col = np.ascontiguousarray(b_c[:, None])
    in_maps = [
        {"embT": embT[c], "W": Wt, "wa": wa_col, "bc": bc_col}
        for c in range(NCORES)
    ]
    res = run_bass_kernel_spmd(nc, in_maps, core_ids=list(range(NCORES)))

    num = np.zeros((B, H), dtype=np.float64)
    e_global = np.empty(N, dtype=np.float32)
    nblk_real = L // BLK
    if TAIL:
        # per-core tail nodes: full forward on the host (tiny)
        for c in range(NCORES):
            g0 = c * SH + L
            et = emb[g0:g0 + TAIL]
            st = np.tanh(et @ W_c.T + b_c) @ w_a
            e_global[g0:g0 + TAIL] = np.exp(st)
            segs = batch[g0:g0 + TAIL]
            for s in np.unique(segs):
                msk = segs == s
                num[s] += e_global[g0:g0 + TAIL][msk] @ et[msk]
    for c in range(NCORES):
        P = res.results[c]["P"]                          # [nmacro, H, NCH]
        e_flat = np.asarray(res.results[c]["e"],
                            dtype=np.float32).reshape(-1)    # [L]
        e_global[c * SH:c * SH + L] = e_flat
        P_flat = np.moveaxis(P, 1, 0).reshape(H, -1)     # [H, L//BLK]
        for b in range(nblk_real):
            g0 = c * SH + BLK * b
            g1 = g0 + BLK
            s0 = batch[g0]
            s1 = batch[g1 - 1]
            if s0 == s1:
                num[s0] += P_flat[:, b]
            else:
                # boundary block: recompute exactly on host per segment run
                segs = batch[g0:g1]
                eb = e_flat[BLK * b: BLK * b + (g1 - g0)].astype(np.float64)
                cuts = np.concatenate(
                    [[0], np.flatnonzero(np.diff(segs)) + 1, [g1 - g0]])
                for r in range(len(cuts) - 1):
                    r0, r1 = cuts[r], cuts[r + 1]
                    num[segs[r0]] += eb[r0:r1] @ emb[g0 + r0: g0 + r1]
    den = np.bincount(batch, weights=e_global, minlength=B)
    den[den == 0.0] = 1.0          # empty segments -> 0 output (matches reference)
    return (num / den[:, None]).astype(np.float32)
